# revision 15
# baseline (speedup 1.0000x reference)
"""Trainium2 Bass kernel for DFlashAttention (qk-norm + neox RoPE + GQA
attention + output projection), tensor-parallel over 8 NeuronCores.

Sharding: 2-way batch DP x 4-way head-group TP.  Core c handles batch
b=c//4 and head group g=c%4 (4 Q heads + 1 KV head).  All activations are
kept feature-major ("transposed", [feat, token]) on chip:

  phase 1: qkv^T = Wqkv_g^T @ X_b^T (PE, fp32r), fused per-head RMSNorm
           (partition sums via ones-matmul) + neox RoPE (DVE).
  phase 2: per head: S^T = K^T-tiles^T.T @ Q^T  (scores transposed so the
           softmax denominator is a partition-dim sum = ones-matmul; no
           max-subtraction needed since |s| <= sqrt(128)), E = exp(S^T)
           (ACT), attn^T = V.T @ E / denom.
  phase 2.5: AllToAll within each 4-core batch group converts head-sharding
           to token-sharding.
  phase 3: out_chunk = attn_chunk^T.T @ Wo (full rows local), disjoint
           [S/4, HIDDEN] output chunk per core; host concatenates.

All matmuls run in float32r (fp32 with 11-bit mantissa, full PE speed at
free-dim >= 256).  Weights/activations are pre-rounded to fp32r on the
host; on-chip matmul operands are produced with float32r output dtype.
"""

import sys

sys.path.insert(0, "/opt/trn_rl_repo")

import numpy as np

import concourse.bass as bass
import concourse.mybir as mybir
import concourse.tile as tile
from concourse import bacc
from concourse.bass_utils import run_bass_kernel_spmd
from concourse.masks import make_identity

f32 = mybir.dt.float32
f32r = mybir.dt.float32r
AF = mybir.ActivationFunctionType
ALU = mybir.AluOpType

HIDDEN = 2048
N_HEADS = 16
N_KV = 4
D = 128
B = 2
S = 2048
EPS = 1e-6
ROPE_BASE = 1000000.0
N_CORES = 8
GROUPS = 4          # head groups == cores per batch group
QH = N_HEADS // GROUPS  # q heads per core (4)

P = 128
KO = HIDDEN // P    # 16 hidden sub-tiles
TC1 = 256           # phase-1 token chunk (fp32r needs >=256 for full speed)


def round_fp32r(x: np.ndarray) -> np.ndarray:
    """Round fp32 to fp32r (RNE to 11-bit mantissa, low 12 bits zero)."""
    u = np.ascontiguousarray(x, dtype=np.float32).view(np.uint32)
    r = u + (0x7FF + ((u >> 12) & 1))
    r &= np.uint32(0xFFFFF000)
    return r.view(np.float32)


def build(s=S):
    """Build the SPMD program for one core (same program on all 8)."""
    assert s % (4 * P) == 0
    QC = s // 4                 # q-chunk == all-to-all token chunk
    assert QC <= 512
    NT = s // P                 # token tiles
    NC1 = s // TC1 if s >= TC1 else 1
    tc1 = min(TC1, s)
    WOC = 256                   # phase-3 Wo column chunk

    nc = bacc.Bacc(None, target_bir_lowering=False, debug=False,
                   num_devices=N_CORES)

    HC = HIDDEN // 2  # each core computes half the output columns
    xt = nc.dram_tensor("xt", [HIDDEN, s], f32r, kind="ExternalInput")
    wqkv = nc.dram_tensor("wqkv", [HIDDEN, (QH + 2) * D], f32r,
                          kind="ExternalInput")
    wo = nc.dram_tensor("wo", [N_HEADS * D, HC], f32r,
                        kind="ExternalInput")
    cos2 = nc.dram_tensor("cos2", [P, s], f32, kind="ExternalInput")
    sinpm = nc.dram_tensor("sinpm", [P, s], f32, kind="ExternalInput")
    qw = nc.dram_tensor("qw", [P, 1], f32, kind="ExternalInput")
    kw = nc.dram_tensor("kw", [P, 1], f32, kind="ExternalInput")
    ones_in = nc.dram_tensor("ones_in", [P, P], f32r, kind="ExternalInput")
    out = nc.dram_tensor("out", [2, QC, HC], f32, kind="ExternalOutput")

    xt_ap = xt.ap().rearrange("(ko p) t -> p ko t", p=P)
    wq_ap = wqkv.ap()[:, 0 : QH * D].rearrange("(ko p) f -> p ko f", p=P)
    wkv_ap = wqkv.ap()[:, QH * D :].rearrange("(ko p) f -> p ko f", p=P)
    wo_ap = wo.ap().rearrange("(ko p) h -> p ko h", p=P)

    rg = [[0, 1, 2, 3, 4, 5, 6, 7]]

    with tile.TileContext(nc) as tc:
        with (
            tc.tile_pool(name="stream", bufs=2) as stream,   # xt / wo chunks
            tc.tile_pool(name="big", bufs=2) as bigp,        # wq,wkv,E,a2a
            tc.tile_pool(name="persist", bufs=1) as persist,
            tc.tile_pool(name="tmp", bufs=2) as tmp,
            tc.tile_pool(name="psum", bufs=1, space="PSUM") as psum,
            tc.tile_pool(name="dram", bufs=1, space="DRAM") as dram,
        ):
            # --- constants / persistent tensors ---
            wq_sb = bigp.tile([P, KO, QH * D], f32r, tag="big")
            nc.sync.dma_start(wq_sb[:], wq_ap)
            wkv_sb = persist.tile([P, KO, 2 * D], f32r, tag="wkv")
            nc.sync.dma_start(wkv_sb[:], wkv_ap)
            cos_sb = persist.tile([P, s], f32, tag="cos")
            nc.sync.dma_start(cos_sb[:], cos2.ap())
            sin_sb = persist.tile([P, s], f32, tag="sin")
            nc.sync.dma_start(sin_sb[:], sinpm.ap())
            qw_sb = persist.tile([P, 1], f32, tag="qw")
            nc.sync.dma_start(qw_sb[:], qw.ap())
            kw_sb = persist.tile([P, 1], f32, tag="kw")
            nc.sync.dma_start(kw_sb[:], kw.ap())
            ones_sb = persist.tile([P, P], f32r, tag="ones")
            nc.sync.dma_start(ones_sb[:], ones_in.ap())
            ident = persist.tile([P, P], f32, tag="ident")
            make_identity(nc, ident)
            epsq_sb = persist.tile([P, 1], f32, tag="epsq")
            nc.vector.memset(epsq_sb[:], EPS * D)
            epsk_sb = persist.tile([P, 1], f32, tag="epsk")
            nc.vector.memset(epsk_sb[:], EPS)

            qr = [persist.tile([P, s], f32r, tag="qr", bufs=5,
                               name=f"qr{h}") for h in range(QH)]
            kr = persist.tile([P, s], f32r, tag="qr", bufs=5)
            vnat = persist.tile([P, NT, D], f32r, tag="vnat")

            # 8 blocks: block j carries my attn chunk for token-chunk j%4
            # (replicated for both quads).  After AllToAll, out[j] = sender
            # j's heads for MY token chunk; j=0..3 are batch 0's head
            # groups, j=4..7 batch 1's — both real data on every core.
            a2a_in = dram.tile([N_CORES, GROUPS, P, QC], f32r)
            a2a_out = dram.tile([N_CORES, GROUPS, P, QC], f32r)

            # ---------------- phase 1: qkv^T + norm + rope ----------------
            for c in range(NC1):
                xt_t = stream.tile([P, KO, tc1], f32r, tag="stream")
                nc.sync.dma_start(xt_t[:], xt_ap[:, :, c * tc1 : (c + 1) * tc1])
                for m in range(QH + 2):
                    ps = psum.tile([P, tc1], f32, tag="mm", bufs=3)
                    for ko in range(KO):
                        if m < QH:
                            lhsT = wq_sb[:, ko, m * D : (m + 1) * D]
                        else:
                            lhsT = wkv_sb[:, ko, (m - QH) * D : (m - QH + 1) * D]
                        nc.tensor.matmul(ps[:], lhsT, xt_t[:, ko, :],
                                         start=(ko == 0), stop=(ko == KO - 1))
                    qsb = tmp.tile([P, tc1], f32, tag="qsb")
                    nc.vector.tensor_copy(qsb[:], ps[:])
                    if m == QH + 1:
                        # v: transpose to natural [token, d] layout
                        for j in range(tc1 // P):
                            pt = psum.tile([P, P], f32, tag="mm", bufs=3)
                            nc.tensor.transpose(pt[:], qsb[:, j * P : (j + 1) * P],
                                                ident[:])
                            nc.vector.tensor_copy(
                                vnat[:, c * (tc1 // P) + j, :], pt[:])
                        continue
                    # q/k: rms-norm (partition sum via ones-matmul) + rope
                    q2 = tmp.tile([P, tc1], f32r, tag="q2")
                    nc.vector.tensor_mul(q2[:], qsb[:], qsb[:])
                    ssp = psum.tile([P, tc1], f32, tag="den", bufs=2)
                    nc.tensor.matmul(ssp[:], ones_sb[:], q2[:],
                                     start=True, stop=True)
                    sq = tmp.tile([P, tc1], f32, tag="sq")
                    if m < QH:
                        # fold the 1/sqrt(D) score scale into rs_q:
                        # rs_q = 1/sqrt(ss + eps*D)  (== 1/sqrt(ms+eps)/sqrt(D))
                        nc.scalar.activation(sq[:], ssp[:], AF.Sqrt,
                                             bias=epsq_sb[:], scale=1.0)
                    else:
                        nc.scalar.activation(sq[:], ssp[:], AF.Sqrt,
                                             bias=epsk_sb[:], scale=1.0 / D)
                    rs = tmp.tile([P, tc1], f32, tag="rs")
                    nc.vector.reciprocal(rs[:], sq[:])
                    qn = tmp.tile([P, tc1], f32, tag="qn")
                    nc.vector.scalar_tensor_tensor(
                        out=qn[:], in0=qsb[:],
                        scalar=(qw_sb[:] if m < QH else kw_sb[:]),
                        in1=rs[:], op0=ALU.mult, op1=ALU.mult)
                    qs_ = tmp.tile([P, tc1], f32, tag="qs")
                    nc.vector.tensor_copy(qs_[0:64], qn[64:128])
                    nc.vector.tensor_copy(qs_[64:128], qn[0:64])
                    csl = cos_sb[:, c * tc1 : (c + 1) * tc1]
                    ssl = sin_sb[:, c * tc1 : (c + 1) * tc1]
                    nc.vector.tensor_mul(qn[:], qn[:], csl)
                    nc.vector.tensor_mul(qs_[:], qs_[:], ssl)
                    dest = qr[m] if m < QH else kr
                    nc.vector.tensor_add(
                        dest[:, c * tc1 : (c + 1) * tc1], qn[:], qs_[:])

            # ---------------- phase 2: attention per (head, q-chunk) -------
            for h in range(QH):
                for qc in range(s // QC):
                    e_t = bigp.tile([P, NT, QC], f32r, tag="big")
                    for kt in range(NT):
                        sp = psum.tile([P, QC], f32, tag="mm", bufs=3)
                        nc.tensor.matmul(
                            sp[:], kr[:, kt * P : (kt + 1) * P],
                            qr[h][:, qc * QC : (qc + 1) * QC],
                            start=True, stop=True)
                        nc.scalar.activation(e_t[:, kt, :], sp[:], AF.Exp)
                    pv = psum.tile([P, QC], f32, tag="pv", bufs=2)
                    for kt in range(NT):
                        nc.tensor.matmul(pv[:], vnat[:, kt, :], e_t[:, kt, :],
                                         start=(kt == 0), stop=(kt == NT - 1))
                    den = psum.tile([P, QC], f32, tag="den", bufs=2)
                    for kt in range(NT):
                        nc.tensor.matmul(den[:], ones_sb[:], e_t[:, kt, :],
                                         start=(kt == 0), stop=(kt == NT - 1))
                    rec = tmp.tile([P, QC], f32, tag="rec")
                    nc.vector.reciprocal(rec[:], den[:])
                    at = tmp.tile([P, QC], f32r, tag="attn")
                    nc.vector.tensor_mul(at[:], pv[:], rec[:])
                    nc.sync.dma_start(a2a_in[qc, h], at[:])
                    nc.sync.dma_start(a2a_in[qc + 4, h], at[:])

            # ---------------- phase 2.5: all-to-all ----------------------
            nc.gpsimd.collective_compute(
                "AllToAll", ALU.bypass, replica_groups=rg,
                ins=[a2a_in.opt()], outs=[a2a_out.opt()])

            # ---------------- phase 3: out = attn_chunk @ Wo_half ---------
            for bb in range(2):
                a2a_sb = bigp.tile([P, N_HEADS, QC], f32r, tag="big",
                                   name=f"a2a_sb{bb}")
                nc.sync.dma_start(
                    a2a_sb[:],
                    a2a_out[4 * bb : 4 * bb + 4].rearrange(
                        "i g d t -> d (i g) t"))
                for hc in range(HC // WOC):
                    wo_t = stream.tile([P, N_HEADS, WOC], f32r, tag="stream")
                    nc.sync.dma_start(wo_t[:],
                                      wo_ap[:, :, hc * WOC : (hc + 1) * WOC])
                    for tt in range(QC // P):
                        op = psum.tile([P, WOC], f32, tag="mm", bufs=3)
                        for ft in range(N_HEADS):
                            nc.tensor.matmul(
                                op[:], a2a_sb[:, ft, tt * P : (tt + 1) * P],
                                wo_t[:, ft, :],
                                start=(ft == 0), stop=(ft == N_HEADS - 1))
                        ob = tmp.tile([P, WOC], f32, tag="ob")
                        nc.vector.tensor_copy(ob[:], op[:])
                        nc.sync.dma_start(
                            out.ap()[bb, tt * P : (tt + 1) * P,
                                     hc * WOC : (hc + 1) * WOC], ob[:])

    nc.finalize()
    return nc


def _rope_tables(positions_b: np.ndarray, s: int):
    """cos2 [128, s] (cos duplicated) and sinpm [128, s] ([-sin; sin])."""
    inv = 1.0 / (ROPE_BASE ** (np.arange(0, D, 2, dtype=np.float64) / D))
    fr = positions_b.astype(np.float64)[:, None] * inv[None, :]  # [s, 64]
    cos = np.cos(fr).T.astype(np.float32)  # [64, s]
    sin = np.sin(fr).T.astype(np.float32)
    cos2 = np.concatenate([cos, cos], 0)
    sinpm = np.concatenate([-sin, sin], 0)
    return np.ascontiguousarray(cos2), np.ascontiguousarray(sinpm)


def _prepare_in_maps(hidden_states, positions, Wqkv, Wo, q_norm_w, k_norm_w,
                     s=S):
    X = np.asarray(hidden_states, dtype=np.float32)
    Wqkv = np.asarray(Wqkv, dtype=np.float32)
    Wo = np.asarray(Wo, dtype=np.float32)
    q_size = N_HEADS * D
    kv_size = N_KV * D

    wo_halves = [round_fp32r(np.ascontiguousarray(Wo[:, :HIDDEN // 2])),
                 round_fp32r(np.ascontiguousarray(Wo[:, HIDDEN // 2:]))]
    ones = np.ones((P, P), dtype=np.float32)
    qw = np.ascontiguousarray(np.asarray(q_norm_w, np.float32)[:, None])
    kw = np.ascontiguousarray(np.asarray(k_norm_w, np.float32)[:, None])

    xts, tabs = [], []
    for b in range(B):
        xts.append(round_fp32r(np.ascontiguousarray(X[b].T)))
        tabs.append(_rope_tables(np.asarray(positions[b]), s))

    in_maps = []
    for c in range(N_CORES):
        b, g = c // GROUPS, c % GROUPS
        wq = Wqkv[:, g * QH * D : (g + 1) * QH * D]
        wk = Wqkv[:, q_size + g * D : q_size + (g + 1) * D]
        wv = Wqkv[:, q_size + kv_size + g * D : q_size + kv_size + (g + 1) * D]
        wl = round_fp32r(np.concatenate([wq, wk, wv], axis=1))
        cos2, sinpm = tabs[b]
        in_maps.append({
            "xt": xts[b], "wqkv": wl, "wo": wo_halves[c // GROUPS],
            "cos2": cos2, "sinpm": sinpm,
            "qw": qw, "kw": kw, "ones_in": ones,
        })
    return in_maps


def _install_profile_hook():
    """Register the axon NTFF profile hook (the agent image's antenv lacks
    axon_hooks; the .so itself supports profiling)."""
    import types
    try:
        import antenv.axon_hooks  # noqa: F401
        return
    except ImportError:
        pass
    try:
        import antenv
        from trn_agent_boot.trn_boot import _ntff_profile_via_ctypes
        hook = _ntff_profile_via_ctypes("/opt/axon/libaxon_pjrt.so")
        mod = types.ModuleType("antenv.axon_hooks")
        mod._hook = hook

        def get_axon_ntff_profile_hook():
            return mod._hook

        def set_axon_ntff_profile_hook(h):
            mod._hook = h

        mod.get_axon_ntff_profile_hook = get_axon_ntff_profile_hook
        mod.set_axon_ntff_profile_hook = set_axon_ntff_profile_hook
        sys.modules["antenv.axon_hooks"] = mod
        antenv.axon_hooks = mod
    except Exception as e:  # profiling is best-effort
        print(f"profile hook install failed: {e}", file=sys.stderr)


_NC_CACHE = {}


def _get_nc(s=S):
    if s not in _NC_CACHE:
        _NC_CACHE[s] = build(s)
    return _NC_CACHE[s]


def run(inputs: dict, trace: bool = False):
    if trace:
        _install_profile_hook()
    in_maps = _prepare_in_maps(**inputs)
    nc = _get_nc()
    res = run_bass_kernel_spmd(nc, in_maps, core_ids=list(range(N_CORES)),
                               trace=trace)
    # core c = (half h0=c//4, token-chunk g=c%4); its "out" is
    # [2 batches, S/4 tokens, HIDDEN/2 columns]
    outs = np.empty((B, S, HIDDEN), dtype=np.float32)
    QC, HC = S // 4, HIDDEN // 2
    for c in range(N_CORES):
        half, g = c // GROUPS, c % GROUPS
        o = res.results[c]["out"]
        for bb in range(B):
            outs[bb, g * QC : (g + 1) * QC,
                 half * HC : (half + 1) * HC] = o[bb]
    return outs, res


def kernel(**inputs) -> np.ndarray:
    out, _ = run(inputs, trace=False)
    return out
